# revision 19
# baseline (speedup 1.0000x reference)
"""FFT-based DCT-II on 8 trn2 NeuronCores (v5: radix 32x128).

Per core (256 rows, no h-split): Makhoul DCT->real-FFT with n = 32*n1' +
n2' (n1' in [0,128), n2' in [0,32)). Stage 1 contracts n1' with K=128
(full PE array, no tiling tricks), producing 128 slots (65 cos + 63 sin
of the 128-point real DFT, cos_64 parked in the sin_0 slot). The mid
transpose roundtrips DRAM with FULLY CONTIGUOUS write legs (t_dram ==
t_sb layout) and 512B-run read legs (r=256 rows stay whole). Stage 2
contracts (m, n2') with K=64, col-tiled pairs; fp16 output with
1KB-contiguous store runs.

Layouts:
  x1[n1', 256 n2' + r] = v[r, 32 n1' + n2']
  slots s = 2a + m: m=0 cos_a (a=0..63), m=1 -sin_a (cos_64 at s=1)
  t_sb = t_dram [128 s, 32 n2', 256 r]
  t2   [64 = (m, n2'), 64 groups a, 256 r]
  y    [16 qq, 2 cg, 2 d, 32 k2', 2 i, 256 r], group a = 4qq + 2i + cg
"""

import numpy as np

N = 4096
R = 2048
RPC = 256

_state = {}


def _tables():
    n1 = np.arange(128, dtype=np.float64)
    f1 = np.zeros((128, 128))
    a_ = np.arange(64, dtype=np.float64)
    f1[:, 0::2] = np.cos(2 * np.pi * n1[:, None] * a_[None, :] / 128)
    f1[:, 3::2] = -np.sin(2 * np.pi * n1[:, None] * a_[None, 1:] / 128)
    f1[:, 1] = np.cos(np.pi * n1)  # cos_64 in the sin_0 slot
    f1_np = f1.astype(np.float16)  # [128, 128]

    n2 = np.arange(32, dtype=np.float64)[:, None]
    k2 = np.arange(32, dtype=np.float64)[None, :]
    hh5 = np.zeros((64, 64, 64))
    for a in range(64):
        for d in range(2):
            k1 = (a if d == 0 else 128 - a) if a >= 1 else (0 if d == 0 else 64)
            kk = 128 * k2 + k1
            th = np.pi * kk * (4 * n2 + 1) / 8192
            cols = (32 * d + np.arange(32))[None, :]
            rows = np.arange(32)[:, None]
            if a == 0:
                hh5[32 * d + rows, 0, cols] = np.cos(th)
            else:
                sgn = 1.0 if d == 0 else -1.0
                hh5[rows, a, cols] = np.cos(th)
                hh5[32 + rows, a, cols] = sgn * np.sin(th)
    hh_np = hh5.astype(np.float16).copy()  # [64, 64, 64]

    k1_map = np.empty(128, dtype=np.int64)
    for a in range(64):
        for d in range(2):
            k1_map[2 * a + d] = (a if d == 0 else 128 - a) if a >= 1 else (
                0 if d == 0 else 64
            )
    return f1_np, hh_np, k1_map


def _build():
    import concourse.tile as tile
    from concourse import bacc, mybir

    f16 = mybir.dt.float16
    f32 = mybir.dt.float32

    nc = bacc.Bacc("TRN2", target_bir_lowering=False, debug=False, num_devices=8)
    x1_d = nc.dram_tensor("x1", [128, 8192], f16, kind="ExternalInput").ap()
    f1_d = nc.dram_tensor("f1", [128, 128], f16, kind="ExternalInput").ap()
    hh_d = nc.dram_tensor("hh", [64, 64, 64], f16, kind="ExternalInput").ap()
    y_d = nc.dram_tensor("y", [16, 2, 2, 32, 2, 256], f16, kind="ExternalOutput").ap()

    with tile.TileContext(nc) as tc:
        with (
            tc.tile_pool(name="const", bufs=1) as const,
            tc.tile_pool(name="data", bufs=1) as data,
            tc.tile_pool(name="dram", bufs=1, space="DRAM") as dram,
            tc.tile_pool(name="ps1", bufs=4, space="PSUM") as ps1,
            tc.tile_pool(name="ps2", bufs=4, space="PSUM") as ps2,
            tc.tile_pool(name="ysb", bufs=4) as ysbp,
        ):
            f1_sb = const.tile([128, 128], f16)
            hh_sb = const.tile([64, 64, 64], f16)
            nc.sync.dma_start(f1_sb[:], f1_d)
            x1_g = []
            for g in range(8):
                xg = data.tile([128, 1024], f16, name=f"x1_{g}")
                nc.sync.dma_start(xg[:], x1_d[:, 1024 * g : 1024 * g + 1024])
                x1_g.append(xg)
            # hh on sync AFTER x (FIFO keeps it off x's bandwidth);
            # write wave 1 goes on scalar so it's not behind hh
            nc.sync.dma_start(hh_sb[:], hh_d)

            t_sb = data.tile([128, 32, 256], f16)
            t_dram = dram.tile([128, 32, 256], f16)  # same layout as t_sb
            t2 = data.tile([64, 64, 256], f16)  # (m n2'), a, r

            # stage 1: per 2-n2' chunk one 1-bank psum tile, one matmul
            # [K=128, M=128, N=512], one copy out.
            cb = 0
            for g in range(8):
                for u in range(2):
                    ps = ps1.tile([128, 512], f32)
                    nc.tensor.matmul(
                        ps[:],
                        f1_sb[:],
                        x1_g[g][:, 512 * u : 512 * u + 512],
                        start=True,
                        stop=True,
                    )
                    n0 = 4 * g + 2 * u
                    dst = t_sb[:, n0 : n0 + 2, :]
                    src = ps[:].rearrange("p (n r) -> p n r", n=2)
                    if cb % 2 == 0:
                        nc.vector.tensor_copy(dst, src)
                    else:
                        nc.scalar.copy(dst, src)
                    cb += 1
                # transpose write legs: fully contiguous, four 8-n2 waves
                # (wave 1/3 on scalar so they're not behind hh on sync)
                if g % 2 == 1:
                    n0 = 8 * (g // 2)
                    d_w = t_dram[:, n0 : n0 + 8, :]
                    s_w = t_sb[:, n0 : n0 + 8, :]
                    if (g // 2) % 2 == 0:
                        nc.scalar.dma_start(d_w, s_w)
                    else:
                        nc.sync.dma_start(d_w, s_w)

            # read legs: t2[32m + n2', a, r] = t_dram[2a + m, n2', r]
            # 512B runs; (j, nh) order so stage-2 j-chunks unblock early
            t_dv = t_dram[:].rearrange("(a m) n r -> a m n r", m=2)
            for j in range(4):
                for nh in range(2):
                    for m in range(2):
                        src = t_dv[
                            16 * j : 16 * j + 16, m, 16 * nh : 16 * nh + 16, :
                        ].rearrange("a n r -> n a r")
                        dst = t2[
                            32 * m + 16 * nh : 32 * m + 16 * nh + 16,
                            16 * j : 16 * j + 16,
                            :,
                        ]
                        if m == 0:
                            nc.sync.dma_start(dst, src)
                        else:
                            nc.scalar.dma_start(dst, src)

            # stage 2: per qq, 4 groups (2 col-tiles x 2 free slots) into
            # one 1-bank psum; copy; contiguous store.
            for qq in range(16):
                ps = ps2.tile([128, 512], f32)
                for i in range(2):
                    for cg in range(2):
                        a = 4 * qq + 2 * i + cg
                        nc.tensor.matmul(
                            ps[64 * cg : 64 * cg + 64, 256 * i : 256 * i + 256],
                            hh_sb[:, a, :],
                            t2[:, a, :],
                            start=True,
                            stop=True,
                        )
                y_sb = ysbp.tile([128, 2, 256], f16)
                src = ps[:].rearrange("p (i r) -> p i r", i=2)
                if qq % 2 == 0:
                    nc.vector.tensor_copy(y_sb[:], src)
                else:
                    nc.scalar.copy(y_sb[:], src)
                dst = y_d[qq].rearrange("c d k i r -> (c d k) i r")
                nc.gpsimd.dma_start(dst, y_sb[:])

    nc.compile()
    return nc


def _pack_x1(x_rows):
    v = np.empty_like(x_rows)
    v[:, : N // 2] = x_rows[:, 0::2]
    v[:, N // 2 :] = x_rows[:, 1::2][:, ::-1]
    x1 = v.reshape(RPC, 128, 32).transpose(1, 2, 0).reshape(128, 8192)
    return np.ascontiguousarray(x1.astype(np.float16))


def kernel(x, _trace: bool = False):
    from concourse.bass_utils import run_bass_kernel_spmd

    x = np.asarray(x, dtype=np.float32)
    assert x.shape == (R, N)
    if "nc" not in _state:
        _state["nc"] = _build()
        _state["tables"] = _tables()
    nc = _state["nc"]
    f1_np, hh_np, k1_map = _state["tables"]

    in_maps = []
    for c in range(8):
        in_maps.append(
            {
                "x1": _pack_x1(x[c * RPC : (c + 1) * RPC]),
                "f1": f1_np,
                "hh": hh_np,
            }
        )

    res = run_bass_kernel_spmd(nc, in_maps, list(range(8)), trace=_trace)

    y = np.empty((R, N), dtype=np.float32)
    for c in range(8):
        ydev = res.results[c]["y"]  # [16, 2, 2, 32, 2, 256]
        perm = np.asarray(ydev, dtype=np.float32).transpose(5, 3, 0, 4, 1, 2)
        perm = perm.reshape(RPC, 32, 128)  # (r, k2', (a d))
        yc = np.empty((RPC, 32, 128), dtype=np.float32)
        yc[:, :, k1_map] = perm
        y[c * RPC : (c + 1) * RPC] = yc.reshape(RPC, N)
    if _trace:
        _state["last_result"] = res
    return y


# revision 20
# speedup vs baseline: 1.1593x; 1.1593x over previous
"""FFT-based DCT-II on 8 trn2 NeuronCores (v5.1: radix 32x128, dual-half).

Per core (256 rows, no h-split): Makhoul DCT->real-FFT with n = 32*n1' +
n2'. Stage 1 contracts n1' with K=128 (full array), 128 slots s = 2a+m
(65 cos + 63 sin, cos_64 parked in the sin_0 slot). Mid transpose
roundtrips DRAM: write legs fully contiguous (t_dram == t_sb layout),
read legs 512B runs into a DUAL-HALF t2 (group-halves on partition
halves -> all 16 SDMA engines + stage-2 row/col quadrant packing, 4
concurrent matmuls). fp16 output, 1KB-contiguous stores.

Layouts:
  x1[n1', 256 n2' + r] = v[r, 32 n1' + n2']
  t_sb = t_dram [128 s=(2a+m), 32 n2', 256 r]
  t2   [128 = (hf, m, n2'), 32 gg, 256 r]   (group a = 32 hf + gg)
  hh   [128 = (hf, m, n2'), 32 gg, 64 = (d, k2')]
  y    [16 qq, 2 hf, 2 d, 32 k2', 2 i, 256 r], a = 32 hf + 2 qq + i
"""

import numpy as np

N = 4096
R = 2048
RPC = 256

_state = {}


def _tables():
    n1 = np.arange(128, dtype=np.float64)
    f1 = np.zeros((128, 128))
    a_ = np.arange(64, dtype=np.float64)
    f1[:, 0::2] = np.cos(2 * np.pi * n1[:, None] * a_[None, :] / 128)
    f1[:, 3::2] = -np.sin(2 * np.pi * n1[:, None] * a_[None, 1:] / 128)
    f1[:, 1] = np.cos(np.pi * n1)  # cos_64 in the sin_0 slot
    f1_np = f1.astype(np.float16)  # [128, 128]

    n2 = np.arange(32, dtype=np.float64)[:, None]
    k2 = np.arange(32, dtype=np.float64)[None, :]
    hh = np.zeros((128, 32, 64))
    for hf in range(2):
        for gg in range(32):
            a = 32 * hf + gg
            for d in range(2):
                k1 = (a if d == 0 else 128 - a) if a >= 1 else (
                    0 if d == 0 else 64
                )
                kk = 128 * k2 + k1
                th = np.pi * kk * (4 * n2 + 1) / 8192
                cols = (32 * d + np.arange(32))[None, :]
                rows = np.arange(32)[:, None]
                base = 64 * hf
                if a == 0:
                    hh[base + 32 * d + rows, gg, cols] = np.cos(th)
                else:
                    sgn = 1.0 if d == 0 else -1.0
                    hh[base + rows, gg, cols] = np.cos(th)
                    hh[base + 32 + rows, gg, cols] = sgn * np.sin(th)
    hh_np = hh.astype(np.float16).copy()  # [128, 32, 64]

    k1_map = np.empty(128, dtype=np.int64)
    for a in range(64):
        for d in range(2):
            k1_map[2 * a + d] = (a if d == 0 else 128 - a) if a >= 1 else (
                0 if d == 0 else 64
            )
    return f1_np, hh_np, k1_map


def _build():
    import concourse.tile as tile
    from concourse import bacc, mybir

    f16 = mybir.dt.float16
    f32 = mybir.dt.float32

    nc = bacc.Bacc("TRN2", target_bir_lowering=False, debug=False, num_devices=8)
    x1_d = nc.dram_tensor("x1", [128, 8192], f16, kind="ExternalInput").ap()
    f1_d = nc.dram_tensor("f1", [128, 128], f16, kind="ExternalInput").ap()
    hh_d = nc.dram_tensor("hh", [128, 32, 64], f16, kind="ExternalInput").ap()
    y_d = nc.dram_tensor(
        "y", [16, 2, 2, 32, 2, 256], f16, kind="ExternalOutput"
    ).ap()

    with tile.TileContext(nc) as tc:
        with (
            tc.tile_pool(name="const", bufs=1) as const,
            tc.tile_pool(name="data", bufs=1) as data,
            tc.tile_pool(name="dram", bufs=1, space="DRAM") as dram,
            tc.tile_pool(name="ps1", bufs=4, space="PSUM") as ps1,
            tc.tile_pool(name="ps2", bufs=4, space="PSUM") as ps2,
            tc.tile_pool(name="ysb", bufs=4) as ysbp,
        ):
            f1_sb = const.tile([128, 128], f16)
            hh_sb = const.tile([128, 32, 64], f16)
            nc.sync.dma_start(f1_sb[:], f1_d)
            x1_g = []
            for g in range(8):
                xg = data.tile([128, 1024], f16, name=f"x1_{g}")
                nc.sync.dma_start(xg[:], x1_d[:, 1024 * g : 1024 * g + 1024])
                x1_g.append(xg)
            # hh on sync AFTER x (FIFO keeps it off x's bandwidth);
            # write waves 1/3 go on scalar so they're not behind hh
            nc.sync.dma_start(hh_sb[:], hh_d)

            t_sb = data.tile([128, 32, 256], f16)
            t_dram = dram.tile([128, 32, 256], f16)  # same layout as t_sb
            t2 = data.tile([128, 32, 256], f16)  # (hf, m, n2'), gg, r

            # stage 1: per 2-n2' chunk one 1-bank psum tile, one matmul
            # [K=128, M=128, N=512], one copy out.
            cb = 0
            for g in range(8):
                for u in range(2):
                    ps = ps1.tile([128, 512], f32)
                    nc.tensor.matmul(
                        ps[:],
                        f1_sb[:],
                        x1_g[g][:, 512 * u : 512 * u + 512],
                        start=True,
                        stop=True,
                    )
                    n0 = 4 * g + 2 * u
                    dst = t_sb[:, n0 : n0 + 2, :]
                    src = ps[:].rearrange("p (n r) -> p n r", n=2)
                    if cb % 2 == 0:
                        nc.vector.tensor_copy(dst, src)
                    else:
                        nc.scalar.copy(dst, src)
                    cb += 1
                # transpose write legs: fully contiguous, four 8-n2 waves
                if g % 2 == 1:
                    n0 = 8 * (g // 2)
                    d_w = t_dram[:, n0 : n0 + 8, :]
                    s_w = t_sb[:, n0 : n0 + 8, :]
                    if (g // 2) % 2 == 0:
                        nc.scalar.dma_start(d_w, s_w)
                    else:
                        nc.sync.dma_start(d_w, s_w)

            # read legs: t2[64 hf + 32 m + n2', gg, r] =
            #            t_dram[2 (32 hf + gg) + m, n2', r]; 512B runs,
            # 32-partition dst, halves split across engine parity.
            t_dv = t_dram[:].rearrange("(a m) n r -> a m n r", m=2)
            for j2 in range(2):
                for hf in range(2):
                    for m in range(2):
                        src = t_dv[
                            32 * hf + 16 * j2 : 32 * hf + 16 * j2 + 16, m, :, :
                        ].rearrange("a n r -> n a r")
                        dst = t2[
                            64 * hf + 32 * m : 64 * hf + 32 * m + 32,
                            16 * j2 : 16 * j2 + 16,
                            :,
                        ]
                        if m == 0:
                            nc.sync.dma_start(dst, src)
                        else:
                            nc.scalar.dma_start(dst, src)

            # stage 2: per qq one 1-bank psum, 4 quadrant-packed matmuls
            # (group halves on row/col groups 0 and 64); copy; store.
            for qq in range(16):
                ps = ps2.tile([128, 512], f32)
                for i in range(2):
                    gg = 2 * qq + i
                    for hf in range(2):
                        nc.tensor.matmul(
                            ps[64 * hf : 64 * hf + 64, 256 * i : 256 * i + 256],
                            hh_sb[64 * hf : 64 * hf + 64, gg, :],
                            t2[64 * hf : 64 * hf + 64, gg, :],
                            start=True,
                            stop=True,
                        )
                y_sb = ysbp.tile([128, 2, 256], f16)
                src = ps[:].rearrange("p (i r) -> p i r", i=2)
                if qq % 2 == 0:
                    nc.vector.tensor_copy(y_sb[:], src)
                else:
                    nc.scalar.copy(y_sb[:], src)
                dst = y_d[qq].rearrange("h d k i r -> (h d k) i r")
                nc.sync.dma_start(dst, y_sb[:])

    nc.compile()
    return nc


def _pack_x1(x_rows):
    v = np.empty_like(x_rows)
    v[:, : N // 2] = x_rows[:, 0::2]
    v[:, N // 2 :] = x_rows[:, 1::2][:, ::-1]
    x1 = v.reshape(RPC, 128, 32).transpose(1, 2, 0).reshape(128, 8192)
    return np.ascontiguousarray(x1.astype(np.float16))


def kernel(x, _trace: bool = False):
    from concourse.bass_utils import run_bass_kernel_spmd

    x = np.asarray(x, dtype=np.float32)
    assert x.shape == (R, N)
    if "nc" not in _state:
        _state["nc"] = _build()
        _state["tables"] = _tables()
    nc = _state["nc"]
    f1_np, hh_np, k1_map = _state["tables"]

    in_maps = []
    for c in range(8):
        in_maps.append(
            {
                "x1": _pack_x1(x[c * RPC : (c + 1) * RPC]),
                "f1": f1_np,
                "hh": hh_np,
            }
        )

    res = run_bass_kernel_spmd(nc, in_maps, list(range(8)), trace=_trace)

    y = np.empty((R, N), dtype=np.float32)
    for c in range(8):
        ydev = res.results[c]["y"]  # [qq, hf, d, k2', i, r]
        perm = np.asarray(ydev, dtype=np.float32).transpose(5, 3, 1, 0, 4, 2)
        perm = perm.reshape(RPC, 32, 128)  # (r, k2', (a d)), a = 32hf+2qq+i
        yc = np.empty((RPC, 32, 128), dtype=np.float32)
        yc[:, :, k1_map] = perm
        y[c * RPC : (c + 1) * RPC] = yc.reshape(RPC, N)
    if _trace:
        _state["last_result"] = res
    return y


# revision 23
# speedup vs baseline: 1.2951x; 1.1171x over previous
"""FFT-based DCT-II on 8 trn2 NeuronCores (v5.1: radix 32x128, dual-half).

Per core (256 rows, no h-split): Makhoul DCT->real-FFT with n = 32*n1' +
n2'. Stage 1 contracts n1' with K=128 (full array), 128 slots s = 2a+m
(65 cos + 63 sin, cos_64 parked in the sin_0 slot). Mid transpose
roundtrips DRAM: write legs fully contiguous (t_dram == t_sb layout),
read legs 512B runs into a DUAL-HALF t2 (group-halves on partition
halves -> all 16 SDMA engines + stage-2 row/col quadrant packing, 4
concurrent matmuls). fp16 output, 1KB-contiguous stores.

Layouts:
  x1[n1', 256 n2' + r] = v[r, 32 n1' + n2']
  t_sb = t_dram [128 s=(2a+m), 32 n2', 256 r]
  t2   [128 = (hf, m, n2'), 32 gg, 256 r]   (group a = 32 hf + gg)
  hh   [128 = (hf, m, n2'), 32 gg, 64 = (d, k2')]
  y    [16 qq, 2 hf, 2 d, 32 k2', 2 i, 256 r], a = 32 hf + 2 qq + i
"""

import numpy as np

N = 4096
R = 2048
RPC = 256

_state = {}


def _tables():
    n1 = np.arange(128, dtype=np.float64)
    f1 = np.zeros((128, 128))
    a_ = np.arange(64, dtype=np.float64)
    f1[:, 0::2] = np.cos(2 * np.pi * n1[:, None] * a_[None, :] / 128)
    f1[:, 3::2] = -np.sin(2 * np.pi * n1[:, None] * a_[None, 1:] / 128)
    f1[:, 1] = np.cos(np.pi * n1)  # cos_64 in the sin_0 slot
    f1_np = f1.astype(np.float16)  # [128, 128]

    n2 = np.arange(32, dtype=np.float64)[:, None]
    k2 = np.arange(32, dtype=np.float64)[None, :]
    hh = np.zeros((128, 32, 64))
    for hf in range(2):
        for gg in range(32):
            a = 32 * hf + gg
            for d in range(2):
                k1 = (a if d == 0 else 128 - a) if a >= 1 else (
                    0 if d == 0 else 64
                )
                kk = 128 * k2 + k1
                th = np.pi * kk * (4 * n2 + 1) / 8192
                cols = (32 * d + np.arange(32))[None, :]
                rows = np.arange(32)[:, None]
                base = 64 * hf
                if a == 0:
                    hh[base + 32 * d + rows, gg, cols] = np.cos(th)
                else:
                    sgn = 1.0 if d == 0 else -1.0
                    hh[base + rows, gg, cols] = np.cos(th)
                    hh[base + 32 + rows, gg, cols] = sgn * np.sin(th)
    hh_np = hh.astype(np.float16).copy()  # [128, 32, 64]

    k1_map = np.empty(128, dtype=np.int64)
    for a in range(64):
        for d in range(2):
            k1_map[2 * a + d] = (a if d == 0 else 128 - a) if a >= 1 else (
                0 if d == 0 else 64
            )
    return f1_np, hh_np, k1_map


def _build():
    import concourse.tile as tile
    from concourse import bacc, mybir

    f16 = mybir.dt.float16
    f32 = mybir.dt.float32

    nc = bacc.Bacc("TRN2", target_bir_lowering=False, debug=False, num_devices=8)
    x1_d = nc.dram_tensor("x1", [128, 8192], f16, kind="ExternalInput").ap()
    f1_d = nc.dram_tensor("f1", [128, 128], f16, kind="ExternalInput").ap()
    hh_d = nc.dram_tensor("hh", [128, 32, 64], f16, kind="ExternalInput").ap()
    y_d = nc.dram_tensor(
        "y", [16, 2, 2, 32, 2, 256], f16, kind="ExternalOutput"
    ).ap()

    with tile.TileContext(nc) as tc:
        with (
            tc.tile_pool(name="const", bufs=1) as const,
            tc.tile_pool(name="data", bufs=1) as data,
            tc.tile_pool(name="dram", bufs=1, space="DRAM") as dram,
            tc.tile_pool(name="ps1", bufs=4, space="PSUM") as ps1,
            tc.tile_pool(name="ps2", bufs=4, space="PSUM") as ps2,
            tc.tile_pool(name="ysb", bufs=8) as ysbp,
        ):
            f1_sb = const.tile([128, 128], f16)
            hh_sb = const.tile([128, 32, 64], f16)
            nc.sync.dma_start(f1_sb[:], f1_d)
            x1_g = []
            for g in range(8):
                xg = data.tile([128, 1024], f16, name=f"x1_{g}")
                nc.sync.dma_start(xg[:], x1_d[:, 1024 * g : 1024 * g + 1024])
                x1_g.append(xg)
            # hh on sync AFTER x (FIFO keeps it off x's bandwidth);
            # write waves 1/3 go on scalar so they're not behind hh
            nc.sync.dma_start(hh_sb[:], hh_d)

            t_sb = data.tile([128, 32, 256], f16)
            t_dram = dram.tile([128, 32, 256], f16)  # same layout as t_sb
            t2 = data.tile([128, 32, 256], f16)  # (hf, m, n2'), gg, r

            # stage 1: per 2-n2' chunk one 1-bank psum tile, one matmul
            # [K=128, M=128, N=512], one copy out.
            cb = 0
            for g in range(8):
                for u in range(2):
                    ps = ps1.tile([128, 512], f32)
                    nc.tensor.matmul(
                        ps[:],
                        f1_sb[:],
                        x1_g[g][:, 512 * u : 512 * u + 512],
                        start=True,
                        stop=True,
                    )
                    n0 = 4 * g + 2 * u
                    dst = t_sb[:, n0 : n0 + 2, :]
                    src = ps[:].rearrange("p (n r) -> p n r", n=2)
                    if cb % 2 == 0:
                        nc.vector.tensor_copy(dst, src)
                    else:
                        nc.scalar.copy(dst, src)
                    cb += 1
                # transpose write legs: fully contiguous, eight 4-n2
                # waves alternating queues (short completion tail)
                n0 = 4 * g
                d_w = t_dram[:, n0 : n0 + 4, :]
                s_w = t_sb[:, n0 : n0 + 4, :]
                if g % 2 == 0:
                    nc.scalar.dma_start(d_w, s_w)
                else:
                    nc.sync.dma_start(d_w, s_w)

            # read legs: t2[64 hf + 32 m + n2', gg, r] =
            #            t_dram[2 (32 hf + gg) + m, n2', r]; 512B runs,
            # 32-partition dst, halves split across engine parity.
            t_dv = t_dram[:].rearrange("(a m) n r -> a m n r", m=2)
            for j2 in range(2):
                for hf in range(2):
                    for m in range(2):
                        src = t_dv[
                            32 * hf + 16 * j2 : 32 * hf + 16 * j2 + 16, m, :, :
                        ].rearrange("a n r -> n a r")
                        dst = t2[
                            64 * hf + 32 * m : 64 * hf + 32 * m + 32,
                            16 * j2 : 16 * j2 + 16,
                            :,
                        ]
                        if m == 0:
                            nc.sync.dma_start(dst, src)
                        else:
                            nc.scalar.dma_start(dst, src)

            # stage 2: per qq one 1-bank psum, 4 quadrant-packed matmuls
            # (group halves on row/col groups 0 and 64); copy; store.
            for qq in range(16):
                ps = ps2.tile([128, 512], f32)
                for i in range(2):
                    gg = 2 * qq + i
                    for hf in range(2):
                        nc.tensor.matmul(
                            ps[64 * hf : 64 * hf + 64, 256 * i : 256 * i + 256],
                            hh_sb[64 * hf : 64 * hf + 64, gg, :],
                            t2[64 * hf : 64 * hf + 64, gg, :],
                            start=True,
                            stop=True,
                        )
                y_sb = ysbp.tile([128, 2, 256], f16)
                src = ps[:].rearrange("p (i r) -> p i r", i=2)
                if qq % 2 == 0:
                    nc.vector.tensor_copy(y_sb[:], src)
                else:
                    nc.scalar.copy(y_sb[:], src)
                dst = y_d[qq].rearrange("h d k i r -> (h d k) i r")
                if qq % 2 == 0:
                    nc.sync.dma_start(dst, y_sb[:])
                else:
                    nc.scalar.dma_start(dst, y_sb[:])

    nc.compile()
    return nc


def _pack_x1(x_rows):
    v = np.empty_like(x_rows)
    v[:, : N // 2] = x_rows[:, 0::2]
    v[:, N // 2 :] = x_rows[:, 1::2][:, ::-1]
    x1 = v.reshape(RPC, 128, 32).transpose(1, 2, 0).reshape(128, 8192)
    return np.ascontiguousarray(x1.astype(np.float16))


def kernel(x, _trace: bool = False):
    from concourse.bass_utils import run_bass_kernel_spmd

    x = np.asarray(x, dtype=np.float32)
    assert x.shape == (R, N)
    if "nc" not in _state:
        _state["nc"] = _build()
        _state["tables"] = _tables()
    nc = _state["nc"]
    f1_np, hh_np, k1_map = _state["tables"]

    in_maps = []
    for c in range(8):
        in_maps.append(
            {
                "x1": _pack_x1(x[c * RPC : (c + 1) * RPC]),
                "f1": f1_np,
                "hh": hh_np,
            }
        )

    res = run_bass_kernel_spmd(nc, in_maps, list(range(8)), trace=_trace)

    y = np.empty((R, N), dtype=np.float32)
    for c in range(8):
        ydev = res.results[c]["y"]  # [qq, hf, d, k2', i, r]
        perm = np.asarray(ydev, dtype=np.float32).transpose(5, 3, 1, 0, 4, 2)
        perm = perm.reshape(RPC, 32, 128)  # (r, k2', (a d)), a = 32hf+2qq+i
        yc = np.empty((RPC, 32, 128), dtype=np.float32)
        yc[:, :, k1_map] = perm
        y[c * RPC : (c + 1) * RPC] = yc.reshape(RPC, N)
    if _trace:
        _state["last_result"] = res
    return y


# revision 24
# speedup vs baseline: 1.3071x; 1.0093x over previous
"""FFT-based DCT-II on 8 trn2 NeuronCores (v5.1: radix 32x128, dual-half).

Per core (256 rows, no h-split): Makhoul DCT->real-FFT with n = 32*n1' +
n2'. Stage 1 contracts n1' with K=128 (full array), 128 slots s = 2a+m
(65 cos + 63 sin, cos_64 parked in the sin_0 slot). Mid transpose
roundtrips DRAM: write legs fully contiguous (t_dram == t_sb layout),
read legs 512B runs into a DUAL-HALF t2 (group-halves on partition
halves -> all 16 SDMA engines + stage-2 row/col quadrant packing, 4
concurrent matmuls). fp16 output, 1KB-contiguous stores.

Layouts:
  x1[n1', 256 n2' + r] = v[r, 32 n1' + n2']
  t_sb = t_dram [128 s=(2a+m), 32 n2', 256 r]
  t2   [128 = (hf, m, n2'), 32 gg, 256 r]   (group a = 32 hf + gg)
  hh   [128 = (hf, m, n2'), 32 gg, 64 = (d, k2')]
  y    [16 qq, 2 hf, 2 d, 32 k2', 2 i, 256 r], a = 32 hf + 2 qq + i
"""

import numpy as np

N = 4096
R = 2048
RPC = 256

_state = {}


def _tables():
    n1 = np.arange(128, dtype=np.float64)
    f1 = np.zeros((128, 128))
    a_ = np.arange(64, dtype=np.float64)
    f1[:, 0::2] = np.cos(2 * np.pi * n1[:, None] * a_[None, :] / 128)
    f1[:, 3::2] = -np.sin(2 * np.pi * n1[:, None] * a_[None, 1:] / 128)
    f1[:, 1] = np.cos(np.pi * n1)  # cos_64 in the sin_0 slot
    f1_np = f1.astype(np.float16)  # [128, 128]

    n2 = np.arange(32, dtype=np.float64)[:, None]
    k2 = np.arange(32, dtype=np.float64)[None, :]
    hh = np.zeros((128, 32, 64))
    for hf in range(2):
        for gg in range(32):
            a = 32 * hf + gg
            for d in range(2):
                k1 = (a if d == 0 else 128 - a) if a >= 1 else (
                    0 if d == 0 else 64
                )
                kk = 128 * k2 + k1
                th = np.pi * kk * (4 * n2 + 1) / 8192
                cols = (32 * d + np.arange(32))[None, :]
                rows = np.arange(32)[:, None]
                base = 64 * hf
                if a == 0:
                    hh[base + 32 * d + rows, gg, cols] = np.cos(th)
                else:
                    sgn = 1.0 if d == 0 else -1.0
                    hh[base + rows, gg, cols] = np.cos(th)
                    hh[base + 32 + rows, gg, cols] = sgn * np.sin(th)
    hh_np = hh.astype(np.float16).copy()  # [128, 32, 64]

    k1_map = np.empty(128, dtype=np.int64)
    for a in range(64):
        for d in range(2):
            k1_map[2 * a + d] = (a if d == 0 else 128 - a) if a >= 1 else (
                0 if d == 0 else 64
            )
    return f1_np, hh_np, k1_map


def _build():
    import concourse.tile as tile
    from concourse import bacc, mybir

    f16 = mybir.dt.float16
    f32 = mybir.dt.float32

    nc = bacc.Bacc("TRN2", target_bir_lowering=False, debug=False, num_devices=8)
    x1_d = nc.dram_tensor("x1", [128, 8192], f16, kind="ExternalInput").ap()
    f1_d = nc.dram_tensor("f1", [128, 128], f16, kind="ExternalInput").ap()
    hh_d = nc.dram_tensor("hh", [128, 32, 64], f16, kind="ExternalInput").ap()
    y_d = nc.dram_tensor(
        "y", [16, 2, 2, 32, 2, 256], f16, kind="ExternalOutput"
    ).ap()

    with tile.TileContext(nc) as tc:
        with (
            tc.tile_pool(name="const", bufs=1) as const,
            tc.tile_pool(name="data", bufs=1) as data,
            tc.tile_pool(name="dram", bufs=1, space="DRAM") as dram,
            tc.tile_pool(name="ps1", bufs=4, space="PSUM") as ps1,
            tc.tile_pool(name="ps2", bufs=4, space="PSUM") as ps2,
            tc.tile_pool(name="ysb", bufs=8) as ysbp,
        ):
            f1_sb = const.tile([128, 128], f16)
            hh_sb = const.tile([128, 32, 64], f16)
            nc.sync.dma_start(f1_sb[:], f1_d)
            x1_g = []
            for g in range(8):
                xg = data.tile([128, 1024], f16, name=f"x1_{g}")
                nc.sync.dma_start(xg[:], x1_d[:, 1024 * g : 1024 * g + 1024])
                x1_g.append(xg)
            # hh on sync AFTER x (FIFO keeps it off x's bandwidth);
            # write waves 1/3 go on scalar so they're not behind hh
            nc.sync.dma_start(hh_sb[:], hh_d)

            t_sb = data.tile([128, 32, 256], f16)
            t_dram = dram.tile([128, 32, 256], f16)  # same layout as t_sb
            t2 = data.tile([128, 32, 256], f16)  # (hf, m, n2'), gg, r

            # stage 1: per 2-n2' chunk one 1-bank psum tile, one matmul
            # [K=128, M=128, N=512], one copy out.
            cb = 0
            for g in range(8):
                for u in range(2):
                    ps = ps1.tile([128, 512], f32)
                    nc.tensor.matmul(
                        ps[:],
                        f1_sb[:],
                        x1_g[g][:, 512 * u : 512 * u + 512],
                        start=True,
                        stop=True,
                    )
                    n0 = 4 * g + 2 * u
                    dst = t_sb[:, n0 : n0 + 2, :]
                    src = ps[:].rearrange("p (n r) -> p n r", n=2)
                    if cb % 2 == 0:
                        nc.vector.tensor_copy(dst, src)
                    else:
                        nc.scalar.copy(dst, src)
                    cb += 1
                # transpose write legs: fully contiguous, eight 4-n2
                # waves, all on sync (SP is idle after the x issues;
                # keeps ACT free for copies)
                n0 = 4 * g
                nc.sync.dma_start(
                    t_dram[:, n0 : n0 + 4, :], t_sb[:, n0 : n0 + 4, :]
                )

            # read legs: t2[64 hf + 32 m + n2', gg, r] =
            #            t_dram[2 (32 hf + gg) + m, n2', r]; 512B runs,
            # 32-partition dst, halves split across engine parity.
            t_dv = t_dram[:].rearrange("(a m) n r -> a m n r", m=2)
            for j2 in range(2):
                for hf in range(2):
                    for m in range(2):
                        src = t_dv[
                            32 * hf + 16 * j2 : 32 * hf + 16 * j2 + 16, m, :, :
                        ].rearrange("a n r -> n a r")
                        dst = t2[
                            64 * hf + 32 * m : 64 * hf + 32 * m + 32,
                            16 * j2 : 16 * j2 + 16,
                            :,
                        ]
                        if m == 0:
                            nc.sync.dma_start(dst, src)
                        else:
                            nc.scalar.dma_start(dst, src)

            # stage 2: per qq one 1-bank psum, 4 quadrant-packed matmuls
            # (group halves on row/col groups 0 and 64); copy; store.
            for qq in range(16):
                ps = ps2.tile([128, 512], f32)
                for i in range(2):
                    gg = 2 * qq + i
                    for hf in range(2):
                        nc.tensor.matmul(
                            ps[64 * hf : 64 * hf + 64, 256 * i : 256 * i + 256],
                            hh_sb[64 * hf : 64 * hf + 64, gg, :],
                            t2[64 * hf : 64 * hf + 64, gg, :],
                            start=True,
                            stop=True,
                        )
                y_sb = ysbp.tile([128, 2, 256], f16)
                src = ps[:].rearrange("p (i r) -> p i r", i=2)
                if qq % 2 == 0:
                    nc.vector.tensor_copy(y_sb[:], src)
                else:
                    nc.scalar.copy(y_sb[:], src)
                dst = y_d[qq].rearrange("h d k i r -> (h d k) i r")
                if qq % 2 == 0:
                    nc.sync.dma_start(dst, y_sb[:])
                else:
                    nc.scalar.dma_start(dst, y_sb[:])

    nc.compile()
    return nc


def _pack_x1(x_rows):
    v = np.empty_like(x_rows)
    v[:, : N // 2] = x_rows[:, 0::2]
    v[:, N // 2 :] = x_rows[:, 1::2][:, ::-1]
    x1 = v.reshape(RPC, 128, 32).transpose(1, 2, 0).reshape(128, 8192)
    return np.ascontiguousarray(x1.astype(np.float16))


def kernel(x, _trace: bool = False):
    from concourse.bass_utils import run_bass_kernel_spmd

    x = np.asarray(x, dtype=np.float32)
    assert x.shape == (R, N)
    if "nc" not in _state:
        _state["nc"] = _build()
        _state["tables"] = _tables()
    nc = _state["nc"]
    f1_np, hh_np, k1_map = _state["tables"]

    in_maps = []
    for c in range(8):
        in_maps.append(
            {
                "x1": _pack_x1(x[c * RPC : (c + 1) * RPC]),
                "f1": f1_np,
                "hh": hh_np,
            }
        )

    res = run_bass_kernel_spmd(nc, in_maps, list(range(8)), trace=_trace)

    y = np.empty((R, N), dtype=np.float32)
    for c in range(8):
        ydev = res.results[c]["y"]  # [qq, hf, d, k2', i, r]
        perm = np.asarray(ydev, dtype=np.float32).transpose(5, 3, 1, 0, 4, 2)
        perm = perm.reshape(RPC, 32, 128)  # (r, k2', (a d)), a = 32hf+2qq+i
        yc = np.empty((RPC, 32, 128), dtype=np.float32)
        yc[:, :, k1_map] = perm
        y[c * RPC : (c + 1) * RPC] = yc.reshape(RPC, N)
    if _trace:
        _state["last_result"] = res
    return y


# revision 26
# speedup vs baseline: 1.3199x; 1.0098x over previous
"""FFT-based DCT-II on 8 trn2 NeuronCores (v5.1: radix 32x128, dual-half).

Per core (256 rows, no h-split): Makhoul DCT->real-FFT with n = 32*n1' +
n2'. Stage 1 contracts n1' with K=128 (full array), 128 slots s = 2a+m
(65 cos + 63 sin, cos_64 parked in the sin_0 slot). Mid transpose
roundtrips DRAM: write legs fully contiguous (t_dram == t_sb layout),
read legs 512B runs into a DUAL-HALF t2 (group-halves on partition
halves -> all 16 SDMA engines + stage-2 row/col quadrant packing, 4
concurrent matmuls). fp16 output, 1KB-contiguous stores.

Layouts:
  x1[n1', 256 n2' + r] = v[r, 32 n1' + n2']
  t_sb = t_dram [128 s=(2a+m), 32 n2', 256 r]
  t2   [128 = (hf, m, n2'), 32 gg, 256 r]   (group a = 32 hf + gg)
  hh   [128 = (hf, m, n2'), 32 gg, 64 = (d, k2')]
  y    [16 qq, 2 hf, 2 d, 32 k2', 2 i, 256 r], a = 32 hf + 2 qq + i
"""

import numpy as np

N = 4096
R = 2048
RPC = 256

_state = {}


def _tables():
    n1 = np.arange(128, dtype=np.float64)
    f1 = np.zeros((128, 128))
    a_ = np.arange(64, dtype=np.float64)
    f1[:, 0::2] = np.cos(2 * np.pi * n1[:, None] * a_[None, :] / 128)
    f1[:, 3::2] = -np.sin(2 * np.pi * n1[:, None] * a_[None, 1:] / 128)
    f1[:, 1] = np.cos(np.pi * n1)  # cos_64 in the sin_0 slot
    f1_np = f1.astype(np.float16)  # [128, 128]

    n2 = np.arange(32, dtype=np.float64)[:, None]
    k2 = np.arange(32, dtype=np.float64)[None, :]
    hh = np.zeros((128, 32, 64))
    for hf in range(2):
        for gg in range(32):
            a = 32 * hf + gg
            for d in range(2):
                k1 = (a if d == 0 else 128 - a) if a >= 1 else (
                    0 if d == 0 else 64
                )
                kk = 128 * k2 + k1
                th = np.pi * kk * (4 * n2 + 1) / 8192
                cols = (32 * d + np.arange(32))[None, :]
                rows = np.arange(32)[:, None]
                base = 64 * hf
                if a == 0:
                    hh[base + 32 * d + rows, gg, cols] = np.cos(th)
                else:
                    sgn = 1.0 if d == 0 else -1.0
                    hh[base + rows, gg, cols] = np.cos(th)
                    hh[base + 32 + rows, gg, cols] = sgn * np.sin(th)
    hh_np = hh.astype(np.float16).copy()  # [128, 32, 64]

    k1_map = np.empty(128, dtype=np.int64)
    for a in range(64):
        for d in range(2):
            k1_map[2 * a + d] = (a if d == 0 else 128 - a) if a >= 1 else (
                0 if d == 0 else 64
            )
    return f1_np, hh_np, k1_map


def _build():
    import concourse.tile as tile
    from concourse import bacc, mybir

    f16 = mybir.dt.float16
    f32 = mybir.dt.float32

    nc = bacc.Bacc("TRN2", target_bir_lowering=False, debug=False, num_devices=8)
    x1_d = nc.dram_tensor("x1", [128, 8192], f16, kind="ExternalInput").ap()
    f1_d = nc.dram_tensor("f1", [128, 128], f16, kind="ExternalInput").ap()
    hh_d = nc.dram_tensor("hh", [128, 32, 64], f16, kind="ExternalInput").ap()
    y_d = nc.dram_tensor(
        "y", [16, 2, 2, 32, 2, 256], f16, kind="ExternalOutput"
    ).ap()

    with tile.TileContext(nc) as tc:
        with (
            tc.tile_pool(name="const", bufs=1) as const,
            tc.tile_pool(name="data", bufs=1) as data,
            tc.tile_pool(name="dram", bufs=1, space="DRAM") as dram,
            tc.tile_pool(name="ps1", bufs=4, space="PSUM") as ps1,
            tc.tile_pool(name="ps2", bufs=4, space="PSUM") as ps2,
            tc.tile_pool(name="ysb", bufs=8) as ysbp,
        ):
            f1_sb = const.tile([128, 128], f16)
            hh_sb = const.tile([128, 32, 64], f16)
            nc.sync.dma_start(f1_sb[:], f1_d)
            x1_g = []
            for g in range(8):
                xg = data.tile([128, 1024], f16, name=f"x1_{g}")
                nc.sync.dma_start(xg[:], x1_d[:, 1024 * g : 1024 * g + 1024])
                x1_g.append(xg)


            t_sb = data.tile([128, 32, 256], f16)
            t_dram = dram.tile([128, 32, 256], f16)  # same layout as t_sb
            t2 = data.tile([128, 32, 256], f16)  # (hf, m, n2'), gg, r

            # stage 1: per 2-n2' chunk one 1-bank psum tile, one matmul
            # [K=128, M=128, N=512], one copy out.
            cb = 0
            for g in range(8):
                for u in range(2):
                    ps = ps1.tile([128, 512], f32)
                    nc.tensor.matmul(
                        ps[:],
                        f1_sb[:],
                        x1_g[g][:, 512 * u : 512 * u + 512],
                        start=True,
                        stop=True,
                    )
                    n0 = 4 * g + 2 * u
                    dst = t_sb[:, n0 : n0 + 2, :]
                    src = ps[:].rearrange("p (n r) -> p n r", n=2)
                    if cb % 2 == 0:
                        nc.vector.tensor_copy(dst, src)
                    else:
                        nc.scalar.copy(dst, src)
                    cb += 1
                # transpose write legs: fully contiguous, eight 4-n2
                # waves, all on sync (SP is idle after the x issues;
                # keeps ACT free for copies)
                n0 = 4 * g
                nc.sync.dma_start(
                    t_dram[:, n0 : n0 + 4, :], t_sb[:, n0 : n0 + 4, :]
                )
                if g == 1:
                    # hh load on scalar, sequenced after two copies so
                    # its 0.5MB transfer misses both the x loads and the
                    # sync queue's write waves; lands well before stage 2
                    nc.scalar.dma_start(hh_sb[:], hh_d)

            # read legs: t2[64 hf + 32 m + n2', gg, r] =
            #            t_dram[2 (32 hf + gg) + m, n2', r]; 512B runs,
            # 32-partition dst, halves split across engine parity.
            t_dv = t_dram[:].rearrange("(a m) n r -> a m n r", m=2)
            for j2 in range(2):
                for hf in range(2):
                    for m in range(2):
                        src = t_dv[
                            32 * hf + 16 * j2 : 32 * hf + 16 * j2 + 16, m, :, :
                        ].rearrange("a n r -> n a r")
                        dst = t2[
                            64 * hf + 32 * m : 64 * hf + 32 * m + 32,
                            16 * j2 : 16 * j2 + 16,
                            :,
                        ]
                        if m == 0:
                            nc.sync.dma_start(dst, src)
                        else:
                            nc.scalar.dma_start(dst, src)

            # stage 2: per qq one 1-bank psum, 4 quadrant-packed matmuls
            # (group halves on row/col groups 0 and 64); copy; store.
            for qq in range(16):
                ps = ps2.tile([128, 512], f32)
                for i in range(2):
                    gg = 2 * qq + i
                    for hf in range(2):
                        nc.tensor.matmul(
                            ps[64 * hf : 64 * hf + 64, 256 * i : 256 * i + 256],
                            hh_sb[64 * hf : 64 * hf + 64, gg, :],
                            t2[64 * hf : 64 * hf + 64, gg, :],
                            start=True,
                            stop=True,
                        )
                y_sb = ysbp.tile([128, 2, 256], f16)
                src = ps[:].rearrange("p (i r) -> p i r", i=2)
                if qq % 2 == 0:
                    nc.vector.tensor_copy(y_sb[:], src)
                else:
                    nc.scalar.copy(y_sb[:], src)
                dst = y_d[qq].rearrange("h d k i r -> (h d k) i r")
                if qq % 2 == 0:
                    nc.sync.dma_start(dst, y_sb[:])
                else:
                    nc.scalar.dma_start(dst, y_sb[:])

    nc.compile()
    return nc


def _pack_x1(x_rows):
    v = np.empty_like(x_rows)
    v[:, : N // 2] = x_rows[:, 0::2]
    v[:, N // 2 :] = x_rows[:, 1::2][:, ::-1]
    x1 = v.reshape(RPC, 128, 32).transpose(1, 2, 0).reshape(128, 8192)
    return np.ascontiguousarray(x1.astype(np.float16))


def kernel(x, _trace: bool = False):
    from concourse.bass_utils import run_bass_kernel_spmd

    x = np.asarray(x, dtype=np.float32)
    assert x.shape == (R, N)
    if "nc" not in _state:
        _state["nc"] = _build()
        _state["tables"] = _tables()
    nc = _state["nc"]
    f1_np, hh_np, k1_map = _state["tables"]

    in_maps = []
    for c in range(8):
        in_maps.append(
            {
                "x1": _pack_x1(x[c * RPC : (c + 1) * RPC]),
                "f1": f1_np,
                "hh": hh_np,
            }
        )

    res = run_bass_kernel_spmd(nc, in_maps, list(range(8)), trace=_trace)

    y = np.empty((R, N), dtype=np.float32)
    for c in range(8):
        ydev = res.results[c]["y"]  # [qq, hf, d, k2', i, r]
        perm = np.asarray(ydev, dtype=np.float32).transpose(5, 3, 1, 0, 4, 2)
        perm = perm.reshape(RPC, 32, 128)  # (r, k2', (a d)), a = 32hf+2qq+i
        yc = np.empty((RPC, 32, 128), dtype=np.float32)
        yc[:, :, k1_map] = perm
        y[c * RPC : (c + 1) * RPC] = yc.reshape(RPC, N)
    if _trace:
        _state["last_result"] = res
    return y
